# revision 3
# baseline (speedup 1.0000x reference)
"""MoE all-to-all dispatch, v3: bf16 device path + exact row balance.

Each core processes exactly N = TK/D = 4096 slots (host moves ownership
surplus to deficit cores and stitches the moved rows back afterwards).
Device: dma_gather xin rows (bf16, 1 KiB each) into SBUF, dma_scatter_add
into a bf16 out buffer.  bf16 halves every DMA stream including the
scatter-add's destination read-modify-write; the host upconverts the
result to f32 exactly (bit shift), so the only error is one f32->bf16
rounding of the payload (~4e-3 rel, budget 2e-2).

Index tensors are host-computed int16 in the SWDGE wrapped layout.  Idx
loads run on the sync engine (HWDGE) so they overlap the gpsimd library
load.
"""

import numpy as np

B, S, H, K = 4, 4096, 512, 2
T = B * S
TK = T * K
D = 8
N = TK // D          # 4096 slots per core
# Variable chunk sizes: small first (pipeline fills fast after the
# library-load gate), small last (short tail drain), big in the middle.
CHS = [128, 128, 512, 512, 512, 512, 512, 512, 512, 256]
assert sum(CHS) == N and all(c % 128 == 0 for c in CHS)
NCH = len(CHS)
OFF = np.cumsum([0] + CHS).tolist()      # slot offsets

TRACE = False
LAST_EXEC_NS = None
LAST_RESULTS = None

_CACHE = {}


def _wrap_idxs16(vals: np.ndarray) -> np.ndarray:
    """SWDGE wrapped int16 layout, per chunk: element i of chunk c at
    [i % 16, OFF[c]//16 + i // 16], replicated across the 8 partition
    groups (128 partitions)."""
    cols = []
    for c in range(NCH):
        seg = vals[OFF[c]:OFF[c + 1]].astype(np.int16)
        cols.append(seg.reshape(CHS[c] // 16, 16).T)
    w = np.concatenate(cols, axis=1)                     # [16, N/16]
    return np.ascontiguousarray(np.tile(w, (8, 1)))      # [128, N/16]


def _build_module():
    from contextlib import ExitStack

    import concourse.bacc as bacc
    import concourse.mybir as mybir
    from concourse.library_config import mlp

    nc = bacc.Bacc("TRN2", debug=False, num_swdge_queues=4,
                   dynamic_dma_scratch_size=65536)
    xin = nc.dram_tensor("xin", [T, H], mybir.dt.bfloat16,
                         kind="ExternalInput")
    sidx = nc.dram_tensor("sidx", [128, N // 16], mybir.dt.int16,
                          kind="ExternalInput")
    didx = nc.dram_tensor("didx", [128, N // 16], mybir.dt.int16,
                          kind="ExternalInput")
    out = nc.dram_tensor("out", [TK, H], mybir.dt.bfloat16,
                         kind="ExternalOutput")

    with (
        nc.Block() as block,
        nc.sbuf_tensor("data", [128, N // 128, H], mybir.dt.bfloat16) as data,
        nc.sbuf_tensor("sidx_sb", [128, N // 16], mybir.dt.int16) as sidx_sb,
        nc.sbuf_tensor("didx_sb", [128, N // 16], mybir.dt.int16) as didx_sb,
        nc.semaphore("io0") as io0,
        nc.semaphore("ssem0") as ssem0,
        nc.semaphore("ssem1") as ssem1,
        nc.semaphore("psem0") as psem0,
        nc.semaphore("psem1") as psem1,
        ExitStack() as stack,
    ):
        psems = (psem0, psem1)
        gsems = [stack.enter_context(nc.semaphore(f"g{c}"))  # noqa: ANT232
                 for c in range(NCH)]
        LOOKAHEAD = 4

        @block.sync
        def _(sync):
            sync.dma_start(sidx_sb[:], sidx[:]).then_inc(io0, 16)
            sync.dma_start(didx_sb[:], didx[:]).then_inc(io0, 16)

        @block.gpsimd
        def _(gpsimd):
            gpsimd.load_library(mlp)

            ssems = (ssem0, ssem1)

            def dslice(c):
                return data[:, OFF[c] // 128:OFF[c + 1] // 128, :]

            def gather(c):
                gpsimd.dma_gather(
                    dslice(c), xin[:],
                    sidx_sb[:, OFF[c] // 16:OFF[c + 1] // 16],
                    CHS[c], CHS[c], H,
                    single_packet=False, queue_num=2 * (c % 2),
                ).then_inc(gsems[c], 16)

            def prep_scatter(c):
                # prepare_only: descgen runs now (no data dependency —
                # descriptors are addresses); the DMA fires at trigger time.
                gpsimd.dma_scatter_add(
                    out[:], dslice(c),
                    didx_sb[:, OFF[c] // 16:OFF[c + 1] // 16],
                    CHS[c], CHS[c], H,
                    single_packet=False, queue_num=1 + 2 * (c % 2),
                    prepare_only=True, sem=ssems[c % 2],
                ).then_inc(psems[c % 2], 1)

            gpsimd.wait_ge(io0, 32)
            GLA, SLA = 4, 2      # gather / scatter-prep lookahead
            for c in range(min(GLA, NCH)):
                gather(c)
            for c in range(min(SLA, NCH)):
                prep_scatter(c)
            for c in range(NCH):
                gpsimd.wait_ge(gsems[c], 16)
                gpsimd.wait_ge(psems[c % 2], c // 2 + 1)
                gpsimd.trigger_dma(1, queue_num=1 + 2 * (c % 2))
                if c + GLA < NCH:
                    gather(c + GLA)
                if c + SLA < NCH:
                    prep_scatter(c + SLA)
            for q in range(2):
                gpsimd.wait_ge(ssems[q], 16 * ((NCH - q + 1) // 2))

    nc.compile()
    return nc


def kernel(input_tensor, expert_indices, expert_mapping):
    global LAST_EXEC_NS, LAST_RESULTS
    import ml_dtypes
    from concourse.bass_utils import run_bass_kernel_spmd

    x32 = np.asarray(input_tensor, dtype=np.float32).reshape(T, H)
    x = x32.astype(ml_dtypes.bfloat16)
    idx = np.asarray(expert_indices, dtype=np.int32).reshape(-1)
    emap = np.asarray(expert_mapping, dtype=np.int32)
    owner = emap[idx]                       # [TK]

    # Balance: every core processes exactly N slots.
    by_owner = [np.nonzero(owner == d)[0] for d in range(D)]
    assigned = []
    surplus = []
    for d in range(D):
        v = by_owner[d]
        assigned.append(v[:N])
        if len(v) > N:
            surplus.append(v[N:])
    surplus = np.concatenate(surplus) if surplus else np.empty(0, np.int64)
    moved = []                              # (core, rows imported by core)
    pos = 0
    for d in range(D):
        need = N - len(assigned[d])
        if need > 0:
            take = surplus[pos:pos + need]
            pos += need
            moved.append((d, take))
            assigned[d] = np.sort(np.concatenate([assigned[d], take]))
    assert pos == len(surplus)

    if "nc" not in _CACHE:
        _CACHE["nc"] = _build_module()
    nc = _CACHE["nc"]

    in_maps = []
    for d in range(D):
        rows = assigned[d]
        in_maps.append({
            "xin": x,
            "sidx": _wrap_idxs16(rows // K),
            "didx": _wrap_idxs16(rows),
        })

    res = run_bass_kernel_spmd(nc, in_maps, list(range(D)), trace=TRACE)
    if TRACE:
        LAST_EXEC_NS = res.exec_time_ns
        LAST_RESULTS = res
    outs = [np.array(res.results[d]["out"]) for d in range(D)]
    for d, rows in moved:
        ow = owner[rows]
        for o in np.unique(ow):
            rr = rows[ow == o]
            outs[o][rr] = outs[d][rr]
        outs[d][rows] = np.zeros((), ml_dtypes.bfloat16)
    stacked = np.stack(outs, axis=0)
    # exact bf16 -> f32 upconvert
    return (stacked.view(np.uint16).astype(np.uint32) << 16).view(np.float32)
